# revision 2
# baseline (speedup 1.0000x reference)
"""Trainium2 Bass kernel for nn_Attention_56487409877769 — restructured v2.

NdLinear-qkv -> 16-head attention -> NdLinear-proj, B=4 N=1024 C=1024 H=16.

Sharding: 8 cores = (batch b, head-group g), b=core//2, g=core%2; core owns
batch b and 8 heads (qkv channel slice 512g:512g+512). Host sums the two
proj channel-partials per batch plus the rank-1 bias term.

v2 changes vs baseline (210us):
  * all matmul operands bf16 (same 1 cyc/row as fp32r at free>=512, but
    legal at ANY free size, halves DMA + SBUF)
  * U = E^T-contract: out [n-tile(128 part), 65] instead of [65, 512] —
    full partition utilization halves U PE time (cost = free size only)
  * softmax denominator lands as column 64 of U (ones column in v), so
    normalize is recip + per-partition tensor_scalar_mul on DVE — no
    partition_broadcast / DMA hop
  * proj is seq-first: Y1^T = O^T-contract @ Wp0^T (K=n), out = Y1-contract
    @ Wp1_g^T (K=c-half): 128 MMs instead of 192
  * B/C/proj software-pipelined at head granularity so ACT's exp stream
    (~66us, the phase-C floor) hides under PE work
"""

import sys

if "/opt/trn_rl_repo" not in sys.path:
    sys.path.insert(0, "/opt/trn_rl_repo")

import numpy as np

B, N, C, H = 4, 1024, 1024, 16
HD = C // H          # 64
SCALE = HD ** -0.5
P = 128
NT = N // P          # 8 partition tiles of the 1024 axes
HPC = 8              # heads per core
VW = HD + 1          # v block width per head: [v | ones] -> 65

_CACHE = {}

LAST_RESULT = None   # test.py reads exec_time_ns / profile off this


def _build(reps=1):
    import concourse.mybir as mybir
    import concourse.tile as tile
    from concourse import bacc

    f32 = mybir.dt.float32
    bf16 = mybir.dt.bfloat16
    fp8 = mybir.dt.float8e4
    Exp = mybir.ActivationFunctionType.Exp
    Add = mybir.AluOpType.add
    DoubleRow = mybir.MatmulPerfMode.DoubleRow

    nc = bacc.Bacc("TRN2", target_bir_lowering=False, debug=False)

    def din(name, shape, dt=f32):
        return nc.dram_tensor(name, shape, dt, kind="ExternalInput").ap()

    x_d = din("x_r", [NT, P, C], bf16)
    wq0t_d = din("wq0t_r", [NT, P, N], bf16)
    wqk1t_d = din("wqk1t_r", [NT, P, 1024], bf16)
    wv1t_d = din("wv1t_r", [NT, P, 512], bf16)
    wp0t_d = din("wp0t_r", [NT, P, N], bf16)
    wp1t_d = din("wp1t_r", [4, P, 1024], bf16)
    bq0_d = din("bq0_rep", [P, N])
    bqk1_d = din("bqk1_t", [P, 8])
    bv1_d = din("bv1_rep", [P, 512])
    out_d = nc.dram_tensor("out_r", [NT, P, C], bf16, kind="ExternalOutput").ap()

    with tile.TileContext(nc) as tc:
      for _rep in range(reps):
        # SBUF pools, opened latest-death-first so release can be LIFO.
        rp_cm = tc.tile_pool(name="rp", bufs=4)
        yp_cm = tc.tile_pool(name="yp", bufs=1)
        wd_cm = tc.tile_pool(name="wd", bufs=1)
        otp_cm = tc.tile_pool(name="otp", bufs=1)
        qkv_cm = tc.tile_pool(name="qkv", bufs=1)
        wb_cm = tc.tile_pool(name="wb", bufs=1)
        ab_cm = tc.tile_pool(name="ab", bufs=1)
        wa_cm = tc.tile_pool(name="wa", bufs=1)
        rp = rp_cm.__enter__()
        yp = yp_cm.__enter__()
        wd = wd_cm.__enter__()
        otp = otp_cm.__enter__()
        qkvp = qkv_cm.__enter__()
        wbp = wb_cm.__enter__()
        abp = ab_cm.__enter__()
        wap = wa_cm.__enter__()

        y1t_sb = yp.tile([P, 4, N], bf16, name="y1t_sb")
        wp0t_sb = wd.tile([P, NT, N], bf16, name="wp0t_sb")
        wp1t_sb = wd.tile([P, 4, 1024], bf16, name="wp1t_sb")
        o_sb = otp.tile([P, NT, HPC, HD], bf16, name="o_sb")
        # q/k in fp8 DoubleRow layout: tile t holds head pair (2t, 2t+1) on
        # partition halves 0:64 / 64:128 (matmul operand bases must be
        # 0/32/64). Free dim [i, n]: block i=0 holds the data, block i=1 is
        # all zeros — DoubleRow contracts 2 rows/partition, the zero block
        # contributes nothing, and the fp8 moving operand still buys the
        # 0.5 cycles/row rate.
        qi_sb = [qkvp.tile([P, 2, N], fp8, name=f"qi{t}_sb") for t in range(4)]
        ki_sb = [qkvp.tile([P, 2, N], fp8, name=f"ki{t}_sb") for t in range(4)]
        vpad_sb = qkvp.tile([P, NT, HPC * VW], bf16, name="vpad_sb")
        ones_sb = qkvp.tile([P, HPC], bf16, name="ones_sb")
        wqk1t_sb = wbp.tile([P, NT, 1024], bf16, name="wqk1t_sb")
        wv1t_sb = wbp.tile([P, NT, 512], bf16, name="wv1t_sb")
        # x1 split by m-half so phase-B chains reading one half don't wait on
        # the other half's bias-adds
        x1t_sb = [abp.tile([P, NT, 512], bf16, name=f"x1t{i}_sb")
                  for i in range(2)]
        bqk1_sb = abp.tile([P, 8], f32, name="bqk1_sb")
        bv1_sb = abp.tile([P, 512], f32, name="bv1_sb")
        x_sb = wap.tile([P, NT, C], bf16, name="x_sb")
        wq0t_sb = wap.tile([P, NT, N], bf16, name="wq0t_sb")
        bq0_sb = wap.tile([P, N], f32, name="bq0_sb")
        zdum_sb = wap.tile([P, 512], bf16, name="zdum_sb")

        # PE warm-up: a zero matmul chain that (1) keeps the PE busy through
        # the ~3.5us launch latency of the first input DMA so phase A starts
        # the moment x arrives, and (2) burns through the p-state ramp so the
        # real chains run at full clock. Result is never read.
        nc.vector.memset(zdum_sb[:], 0.0)

        # ---------------- DMA issue (SP queue, transfer order = emission) ----
        # A-critical first: x k-tiles and wq0t m-halves interleaved so the
        # first A chain's operands land earliest.
        for t in range(NT):
            nc.sync.dma_start(out=x_sb[:, t], in_=x_d[t])
            nc.sync.dma_start(out=wq0t_sb[:, t, 0:512], in_=wq0t_d[t, :, 0:512])
            if t == 0:
                nc.sync.dma_start(out=bq0_sb[:], in_=bq0_d)
                nc.sync.dma_start(out=bqk1_sb[:], in_=bqk1_d)
                nc.sync.dma_start(out=bv1_sb[:], in_=bv1_d)
        for t in range(NT):
            nc.sync.dma_start(
                out=wq0t_sb[:, t, 512:1024], in_=wq0t_d[t, :, 512:1024])
        for t in range(NT):
            nc.sync.dma_start(out=wqk1t_sb[:, t], in_=wqk1t_d[t])
        for t in range(NT):
            nc.sync.dma_start(out=wv1t_sb[:, t], in_=wv1t_d[t])
        for t in range(NT):
            nc.sync.dma_start(out=wp0t_sb[:, t], in_=wp0t_d[t])
        for t in range(4):
            nc.sync.dma_start(out=wp1t_sb[:, t], in_=wp1t_d[t])

        # Pool engine: build the ones columns of vpad and the zero DoubleRow
        # blocks of q/k during phase A.
        nc.gpsimd.memset(ones_sb[:], 1.0)
        for t in range(4):
            nc.gpsimd.memset(qi_sb[t][:, 1, :], 0.0)
            nc.gpsimd.memset(ki_sb[t][:, 1, :], 0.0)
        for t in range(NT):
            od = vpad_sb[:, t, :].rearrange(
                "p (h j) -> p h j", h=HPC)[:, :, HD:VW]
            nc.gpsimd.tensor_copy(od, ones_sb[:, :, None])

        # ---------------- phase A: x1T = (Wq0 @ x[b])^T --------------------
        psa_cm = tc.tile_pool(name="psa", bufs=8, space="PSUM")
        psa = psa_cm.__enter__()
        ps_warm = psa.tile([P, 512], f32, tag="psa", name="ps_warm")
        for k in range(10):
            nc.tensor.matmul(
                ps_warm[:], zdum_sb[:, 0:128], zdum_sb[:],
                start=(k == 0), stop=(k == 9),
            )
        for mc in range(2):
            msl = slice(512 * mc, 512 * mc + 512)
            for ct in range(NT):
                ps = psa.tile([P, 512], f32, tag="psa", name="ps_a")
                for k in range(NT):
                    nc.tensor.matmul(
                        ps[:],
                        x_sb[:, k, 128 * ct:128 * ct + 128],
                        wq0t_sb[:, k, msl],
                        start=(k == 0), stop=(k == NT - 1),
                    )
                nc.vector.tensor_tensor(
                    x1t_sb[mc][:, ct, :], ps[:], bq0_sb[:, msl], Add)
        psa_cm.__exit__(None, None, None)
        wa_cm.__exit__(None, None, None)

        # E buffers reuse the A-phase input space (opened after wa closes).
        # 5 rotating buffers: exp head h writes e_tiles[h % 5].
        ep_cm = tc.tile_pool(name="ep", bufs=1)
        ep = ep_cm.__enter__()
        EB = 5
        e_tiles = [ep.tile([P, NT, N], bf16, name=f"e{i}_sb") for i in range(EB)]

        # ---------------- phase B chains (emitted interleaved with C) -------
        pst_cm = tc.tile_pool(name="pst", bufs=2, space="PSUM")
        psb_cm = tc.tile_pool(name="psb", bufs=4, space="PSUM")
        pst = pst_cm.__enter__()
        psb = psb_cm.__enter__()

        # dt 0-3: q head pair dt; dt 4-7: k head pair dt-4
        def qk_chain(dt, nch):
            nsl = slice(512 * nch, 512 * nch + 512)
            ps = psb.tile([P, 512], f32, tag="psb", name="ps_b")
            for k in range(NT):
                nc.tensor.matmul(
                    ps[:],
                    wqk1t_sb[:, k, 128 * dt:128 * dt + 128],
                    x1t_sb[nch][:, k, :],
                    start=(k == 0), stop=(k == NT - 1),
                )
            dst = qi_sb[dt] if dt < 4 else ki_sb[dt - 4]
            nc.vector.tensor_scalar_add(
                dst[:, 0, nsl], ps[:], bqk1_sb[:, dt:dt + 1])

        def v_chain(mt):
            ps = psb.tile([P, 512], f32, tag="psb", name="ps_v")
            for k in range(NT):
                nc.tensor.matmul(
                    ps[:],
                    x1t_sb[mt // 4][:, k, 128 * (mt % 4):128 * (mt % 4) + 128],
                    wv1t_sb[:, k, :],
                    start=(k == 0), stop=(k == NT - 1),
                )
            vdst = vpad_sb[:, mt, :].rearrange(
                "p (h j) -> p h j", h=HPC)[:, :, 0:HD]
            vsrc = ps[:].rearrange("p (h j) -> p h j", h=HPC)
            bsrc = bv1_sb[:].rearrange("p (h j) -> p h j", h=HPC)
            nc.vector.tensor_tensor(vdst, vsrc, bsrc, Add)

        # ---------------- phase C helpers -----------------------------------
        def sc_chunk(h, mt):
            t, pr = h // 2, slice(64 * (h % 2), 64 * (h % 2) + 64)
            st = pst.tile([P, N], f32, tag="st", name="ps_st")
            for nch in range(2):
                nsl = slice(512 * nch, 512 * nch + 512)
                nc.tensor.matmul(
                    st[:, nsl],
                    ki_sb[t][pr, :, 128 * mt:128 * mt + 128],
                    qi_sb[t][pr, :, nsl],
                    start=True, stop=True,
                    perf_mode=DoubleRow,
                )
            nc.scalar.activation(
                e_tiles[h % EB][:, mt, :], st[:], Exp, scale=SCALE)

        psu_cm = psp1_cm = None
        psu = psp1 = None

        def u_half(h, half):
            u = psu.tile([P, 512], f32, tag="u", name="ps_u")
            et = e_tiles[h % EB]
            for jj in range(4):
                nt = 4 * half + jj
                usl = slice(65 * jj, 65 * jj + 65)
                for mt in range(NT):
                    nc.tensor.matmul(
                        u[:, usl],
                        et[:, mt, 128 * nt:128 * nt + 128],
                        vpad_sb[:, mt, VW * h:VW * h + VW],
                        start=(mt == 0), stop=(mt == NT - 1),
                    )
            for jj in range(4):
                nt = 4 * half + jj
                r = rp.tile([P, 1], f32, tag="r", name="r_sb")
                nc.vector.reciprocal(r[:], u[:, 65 * jj + 64:65 * jj + 65])
                nc.vector.tensor_scalar_mul(
                    o_sb[:, nt, h, :], u[:, 65 * jj:65 * jj + 64], r[:])

        def p1_chain(pair, mh):
            ps = psp1.tile([P, 512], f32, tag="p1", name="ps_y")
            for k in range(NT):
                nc.tensor.matmul(
                    ps[:],
                    o_sb[:, k, 2 * pair:2 * pair + 2, :],
                    wp0t_sb[:, k, 512 * mh:512 * mh + 512],
                    start=(k == 0), stop=(k == NT - 1),
                )
            nc.vector.tensor_copy(
                y1t_sb[:, pair, 512 * mh:512 * mh + 512], ps[:])

        # ---------------- interleaved B + C schedule ------------------------
        # head h's scores need q pair chain dt=h//2 and k pair chain 4+h//2
        # (k nch=0 covers score mt 0-3, nch=1 mt 4-7). U of head h is emitted
        # in score block h+4 so exp h+5 (EB=5 rotation, same E buffer) never
        # waits on pending reads. proj1 of pair p follows U of head 2p+1.
        qk_chain(0, 0)
        qk_chain(0, 1)
        qk_chain(4, 0)
        for mt in range(4):
            sc_chunk(0, mt)
        qk_chain(4, 1)
        for mt in range(4, NT):
            sc_chunk(0, mt)
        qk_chain(1, 0)
        qk_chain(1, 1)

        fillers = {
            1: [lambda: qk_chain(5, 0), lambda: qk_chain(5, 1),
                lambda: v_chain(0), lambda: v_chain(1)],
            2: [lambda: v_chain(3), lambda: v_chain(4),
                lambda: v_chain(5), lambda: v_chain(6)],
            3: [lambda: qk_chain(2, 0), lambda: qk_chain(2, 1),
                lambda: qk_chain(6, 0), lambda: qk_chain(6, 1)],
            4: [lambda: u_half(0, 0), lambda: u_half(0, 1),
                lambda: u_half(1, 0), lambda: u_half(1, 1)],
            5: [lambda: u_half(2, 0), lambda: u_half(2, 1),
                lambda: p1_chain(0, 0), lambda: p1_chain(0, 1)],
            6: [lambda: u_half(3, 0), lambda: u_half(3, 1),
                lambda: u_half(4, 0), lambda: u_half(4, 1)],
            7: [lambda: p1_chain(1, 0), lambda: p1_chain(1, 1),
                lambda: u_half(5, 0), lambda: u_half(5, 1)],
        }

        for h in range(1, HPC):
            fl = fillers[h]
            for mt in range(NT):
                sc_chunk(h, mt)
                if mt % 2 == 1:
                    fl[mt // 2]()
            if h == 1:
                v_chain(2)
            elif h == 2:
                v_chain(7)
            elif h == 3:
                # last B chains, then swap psb banks for psu/psp1
                qk_chain(3, 0)
                qk_chain(3, 1)
                qk_chain(7, 0)
                qk_chain(7, 1)
                psb_cm.__exit__(None, None, None)
                psu_cm = tc.tile_pool(name="psu", bufs=2, space="PSUM")
                psp1_cm = tc.tile_pool(name="psp1", bufs=2, space="PSUM")
                psu = psu_cm.__enter__()
                psp1 = psp1_cm.__enter__()

        # C tail: remaining U and proj1
        u_half(6, 0); u_half(6, 1)
        p1_chain(2, 0); p1_chain(2, 1)
        u_half(7, 0); u_half(7, 1)
        p1_chain(3, 0); p1_chain(3, 1)

        psp1_cm.__exit__(None, None, None)
        psu_cm.__exit__(None, None, None)
        pst_cm.__exit__(None, None, None)

        # ---------------- proj2: out = Y1 @ Wp1_g^T -------------------------
        psp2_cm = tc.tile_pool(name="psp2", bufs=6, space="PSUM")
        psp2 = psp2_cm.__enter__()
        for mt in range(NT):
            for dch in range(2):
                dsl = slice(512 * dch, 512 * dch + 512)
                ps = psp2.tile([P, 512], f32, tag="p2", name="ps_o")
                for k in range(4):
                    nc.tensor.matmul(
                        ps[:],
                        y1t_sb[:, k, 128 * mt:128 * mt + 128],
                        wp1t_sb[:, k, dsl],
                        start=(k == 0), stop=(k == 3),
                    )
                ostage = rp.tile([P, 512], bf16, tag="ost", name="out_stage",
                                 bufs=6)
                if dch == 0:
                    nc.vector.tensor_copy(ostage[:], ps[:])
                else:
                    nc.scalar.copy(ostage[:], ps[:])
                nc.sync.dma_start(out=out_d[mt, :, dsl], in_=ostage[:])
        psp2_cm.__exit__(None, None, None)

        ep_cm.__exit__(None, None, None)
        ab_cm.__exit__(None, None, None)
        wb_cm.__exit__(None, None, None)
        qkv_cm.__exit__(None, None, None)
        otp_cm.__exit__(None, None, None)
        wd_cm.__exit__(None, None, None)
        yp_cm.__exit__(None, None, None)
        rp_cm.__exit__(None, None, None)

    nc.compile()
    return nc


def _get_nc(reps=1):
    key = ("nc", reps)
    if key not in _CACHE:
        _CACHE[key] = _build(reps)
    return _CACHE[key]


def _in_maps(x, Wq0, bq0, Wq1, bq1, Wp0, bp0, Wp1, bp1):
    import ml_dtypes
    f = np.float32
    bf = ml_dtypes.bfloat16
    x = np.asarray(x, f)
    Wq0 = np.asarray(Wq0, f); bq0 = np.asarray(bq0, f)
    Wq1 = np.asarray(Wq1, f); bq1 = np.asarray(bq1, f)
    Wp0 = np.asarray(Wp0, f); Wp1 = np.asarray(Wp1, f)
    wq0t = np.ascontiguousarray(Wq0.T.reshape(NT, P, N)).astype(bf)
    wp0t = np.ascontiguousarray(Wp0.T.reshape(NT, P, N)).astype(bf)
    bq0r = np.ascontiguousarray(np.broadcast_to(bq0, (P, N)))
    maps = []
    for core in range(8):
        b, g = core // 2, core % 2
        qs = slice(512 * g, 512 * g + 512)
        ks = slice(1024 + 512 * g, 1024 + 512 * g + 512)
        vs = slice(2048 + 512 * g, 2048 + 512 * g + 512)
        wqk1 = np.concatenate([Wq1[qs], Wq1[ks]], 0)          # (1024 d', 1024 c)
        m = {
            "x_r": np.ascontiguousarray(x[b].reshape(NT, P, C)).astype(bf),
            "wq0t_r": wq0t,
            "wqk1t_r": np.ascontiguousarray(
                wqk1.T.reshape(NT, P, 1024)).astype(bf),
            "wv1t_r": np.ascontiguousarray(
                Wq1[vs].T.reshape(NT, P, 512)).astype(bf),
            "wp0t_r": wp0t,
            "wp1t_r": np.ascontiguousarray(
                Wp1[:, qs].T.reshape(4, P, 1024)).astype(bf),
            "bq0_rep": bq0r,
            "bqk1_t": np.ascontiguousarray(
                np.concatenate([bq1[qs], bq1[ks]]).reshape(8, P).T),
            "bv1_rep": np.ascontiguousarray(np.broadcast_to(bq1[vs], (P, 512))),
        }
        maps.append(m)
    return maps


def kernel(x, Wq0, bq0, Wq1, bq1, Wp0, bp0, Wp1, bp1):
    global LAST_RESULT
    import os

    # The SPMD execute path needs jax's axon PJRT backend; a harness that
    # pinned JAX_PLATFORMS=cpu (common for running the jax reference) would
    # otherwise hide the NeuronCores from this process.
    if "axon" not in os.environ.get("JAX_PLATFORMS", "axon"):
        os.environ.pop("JAX_PLATFORMS", None)
    # This container lacks antenv.axon_hooks, so the BASS_TRACE=1 NTFF path
    # in run_bass_kernel_spmd raises ModuleNotFoundError. Force tracing off
    # (a crash would otherwise replace a working run).
    os.environ["BASS_NEVER_TRACE"] = "1"
    from concourse.bass_utils import run_bass_kernel_spmd

    nc = _get_nc()
    maps = _in_maps(x, Wq0, bq0, Wq1, bq1, Wp0, bp0, Wp1, bp1)
    res = run_bass_kernel_spmd(nc, maps, list(range(8)))
    LAST_RESULT = res
    parts = [np.asarray(r["out_r"]).astype(np.float32).reshape(N, C)
             for r in res.results]
    f = np.float32
    bp0 = np.asarray(bp0, f); bp1 = np.asarray(bp1, f)
    Wp1 = np.asarray(Wp1, f)
    bias = np.outer(bp0, Wp1.sum(axis=1)) + bp1[None, :]
    out = np.stack(
        [parts[2 * b] + parts[2 * b + 1] + bias for b in range(B)], 0)
    return out.astype(f)


# revision 3
# speedup vs baseline: 1.0958x; 1.0958x over previous
"""Trainium2 Bass kernel for nn_Attention_56487409877769 — restructured v2.

NdLinear-qkv -> 16-head attention -> NdLinear-proj, B=4 N=1024 C=1024 H=16.

Sharding: 8 cores = (batch b, head-group g), b=core//2, g=core%2; core owns
batch b and 8 heads (qkv channel slice 512g:512g+512). Host sums the two
proj channel-partials per batch plus the rank-1 bias term.

Design (210us baseline -> ~136us):
  * all matmul operands bf16 (same 1 cyc/row as fp32r at free>=512, but
    legal at ANY free size, halves DMA + SBUF); input x, all weights and
    the output staging are bf16, PSUM accumulation stays fp32
  * scores run as fp8(e4m3) DoubleRow matmuls at 0.5 cyc/row: q/k tiles
    hold a head pair on partition halves 0:64/64:128 with free dim
    [2, n] whose second contraction block is zeros (matmul operand base
    partitions must be 0/32/64, so 2x32-packing is not available)
  * U = E^T-contract: out [n-tile(128 part), 65] instead of [65, 512] —
    full partition utilization halves U PE time (cost = free size only);
    softmax denominator lands as column 64 (ones column in v), normalize
    is recip + per-partition tensor_scalar_mul on DVE
  * proj is seq-first: Y1^T = O^T-contract @ Wp0^T (K=n), out = Y1-contract
    @ Wp1_g^T (K=c-half): 128 MMs instead of 192
  * B/C/proj software-pipelined at head granularity: the ACT exp stream
    (~67us, the hard phase-C floor — exp is ACT-only and has no 16-bit
    speedup) runs dense from ~38us; PSUM pools never swap banks
    mid-kernel (a new pool inherits write-after-read waits against the
    lagging exp stream); E buffers rotate 6-deep (same-block reuse
    corrupts results on device); a zero-matmul warmup chain covers the
    first DMA's launch latency and the PE p-state ramp
  * tail: p1(3)'s k0-3 steps hide before u(7,1); its copies go through
    the by-then-idle ACT engine; proj2 streams straight to bf16 out
    staging and DMA

Per-core PE busy ~126us of the ~136us wall; ACT exp ends ~109us.
"""

import sys

if "/opt/trn_rl_repo" not in sys.path:
    sys.path.insert(0, "/opt/trn_rl_repo")

import numpy as np

B, N, C, H = 4, 1024, 1024, 16
HD = C // H          # 64
SCALE = HD ** -0.5
P = 128
NT = N // P          # 8 partition tiles of the 1024 axes
HPC = 8              # heads per core
VW = HD + 1          # v block width per head: [v | ones] -> 65

_CACHE = {}

LAST_RESULT = None   # test.py reads exec_time_ns / profile off this


def _build(reps=1):
    import concourse.mybir as mybir
    import concourse.tile as tile
    from concourse import bacc

    f32 = mybir.dt.float32
    bf16 = mybir.dt.bfloat16
    fp8 = mybir.dt.float8e4
    Exp = mybir.ActivationFunctionType.Exp
    Copy = mybir.ActivationFunctionType.Copy
    Add = mybir.AluOpType.add
    DoubleRow = mybir.MatmulPerfMode.DoubleRow

    nc = bacc.Bacc("TRN2", target_bir_lowering=False, debug=False)

    def din(name, shape, dt=f32):
        return nc.dram_tensor(name, shape, dt, kind="ExternalInput").ap()

    x_d = din("x_r", [NT, P, C], bf16)
    wq0t_d = din("wq0t_r", [NT, P, N], bf16)
    wqk1t_d = din("wqk1t_r", [NT, P, 1024], bf16)
    wv1t_d = din("wv1t_r", [NT, P, 512], bf16)
    wp0t_d = din("wp0t_r", [NT, P, N], bf16)
    wp1t_d = din("wp1t_r", [4, P, 1024], bf16)
    bq0_d = din("bq0_rep", [P, N])
    bqk1_d = din("bqk1_t", [P, 8])
    bv1_d = din("bv1_rep", [P, 512])
    out_d = nc.dram_tensor("out_r", [NT, P, C], bf16, kind="ExternalOutput").ap()

    with tile.TileContext(nc) as tc:
      for _rep in range(reps):
        # SBUF pools, opened latest-death-first so release can be LIFO.
        rp_cm = tc.tile_pool(name="rp", bufs=4)
        yp_cm = tc.tile_pool(name="yp", bufs=1)
        wd_cm = tc.tile_pool(name="wd", bufs=1)
        otp_cm = tc.tile_pool(name="otp", bufs=1)
        qkv_cm = tc.tile_pool(name="qkv", bufs=1)
        wb_cm = tc.tile_pool(name="wb", bufs=1)
        ab_cm = tc.tile_pool(name="ab", bufs=1)
        wa_cm = tc.tile_pool(name="wa", bufs=1)
        rp = rp_cm.__enter__()
        yp = yp_cm.__enter__()
        wd = wd_cm.__enter__()
        otp = otp_cm.__enter__()
        qkvp = qkv_cm.__enter__()
        wbp = wb_cm.__enter__()
        abp = ab_cm.__enter__()
        wap = wa_cm.__enter__()

        y1t_sb = yp.tile([P, 4, N], bf16, name="y1t_sb")
        wp0t_sb = wd.tile([P, NT, N], bf16, name="wp0t_sb")
        wp1t_sb = wd.tile([P, 4, 1024], bf16, name="wp1t_sb")
        o_sb = otp.tile([P, NT, HPC, HD], bf16, name="o_sb")
        # q/k in fp8 DoubleRow layout: tile t holds head pair (2t, 2t+1) on
        # partition halves 0:64 / 64:128 (matmul operand bases must be
        # 0/32/64). Free dim [i, n]: block i=0 holds the data, block i=1 is
        # all zeros — DoubleRow contracts 2 rows/partition, the zero block
        # contributes nothing, and the fp8 moving operand still buys the
        # 0.5 cycles/row rate.
        qi_sb = [qkvp.tile([P, 2, N], fp8, name=f"qi{t}_sb") for t in range(4)]
        ki_sb = [qkvp.tile([P, 2, N], fp8, name=f"ki{t}_sb") for t in range(4)]
        vpad_sb = qkvp.tile([P, NT, HPC * VW], bf16, name="vpad_sb")
        ones_sb = qkvp.tile([P, HPC], bf16, name="ones_sb")
        wqk1t_sb = wbp.tile([P, NT, 1024], bf16, name="wqk1t_sb")
        wv1t_sb = wbp.tile([P, NT, 512], bf16, name="wv1t_sb")
        # x1 split by m-half so phase-B chains reading one half don't wait on
        # the other half's bias-adds
        x1t_sb = [abp.tile([P, NT, 512], bf16, name=f"x1t{i}_sb")
                  for i in range(2)]
        bqk1_sb = abp.tile([P, 8], f32, name="bqk1_sb")
        bv1_sb = abp.tile([P, 512], f32, name="bv1_sb")
        x_sb = wap.tile([P, NT, C], bf16, name="x_sb")
        wq0t_sb = wap.tile([P, NT, N], bf16, name="wq0t_sb")
        bq0_sb = wap.tile([P, N], f32, name="bq0_sb")
        zdum_sb = wap.tile([P, 512], bf16, name="zdum_sb")

        # PE warm-up: a zero matmul chain that (1) keeps the PE busy through
        # the ~3.5us launch latency of the first input DMA so phase A starts
        # the moment x arrives, and (2) burns through the p-state ramp so the
        # real chains run at full clock. Result is never read.
        nc.gpsimd.memset(zdum_sb[:], 0.0)

        # ---------------- DMA issue (SP queue, transfer order = emission) ----
        # A-critical first: x k-tiles and wq0t m-halves interleaved so the
        # first A chain's operands land earliest.
        for t in range(NT):
            nc.sync.dma_start(out=x_sb[:, t], in_=x_d[t])
            nc.sync.dma_start(out=wq0t_sb[:, t, 0:512], in_=wq0t_d[t, :, 0:512])
        # bias loads: after the A-critical x/w-h0 stream (which feeds the
        # first 8 chains back-to-back), before w-h1; first bias-add needs
        # bq0 at ~t+13us
        nc.sync.dma_start(out=bq0_sb[:], in_=bq0_d)
        nc.sync.dma_start(out=bqk1_sb[:], in_=bqk1_d)
        nc.sync.dma_start(out=bv1_sb[:], in_=bv1_d)
        for t in range(NT):
            nc.sync.dma_start(
                out=wq0t_sb[:, t, 512:1024], in_=wq0t_d[t, :, 512:1024])
        for t in range(NT):
            nc.sync.dma_start(out=wqk1t_sb[:, t], in_=wqk1t_d[t])
        for t in range(NT):
            nc.sync.dma_start(out=wv1t_sb[:, t], in_=wv1t_d[t])
        for t in range(NT):
            nc.sync.dma_start(out=wp0t_sb[:, t], in_=wp0t_d[t])
        for t in range(4):
            nc.sync.dma_start(out=wp1t_sb[:, t], in_=wp1t_d[t])

        # Pool engine: build the ones columns of vpad and the zero DoubleRow
        # blocks of q/k during phase A.
        nc.gpsimd.memset(ones_sb[:], 1.0)
        for t in range(4):
            nc.gpsimd.memset(qi_sb[t][:, 1, :], 0.0)
            nc.gpsimd.memset(ki_sb[t][:, 1, :], 0.0)
        for t in range(NT):
            od = vpad_sb[:, t, :].rearrange(
                "p (h j) -> p h j", h=HPC)[:, :, HD:VW]
            nc.gpsimd.tensor_copy(od, ones_sb[:, :, None])

        # ---------------- phase A: x1T = (Wq0 @ x[b])^T --------------------
        psa_cm = tc.tile_pool(name="psa", bufs=8, space="PSUM")
        psa = psa_cm.__enter__()
        ps_warm = psa.tile([P, 512], f32, tag="psa", name="ps_warm")
        for k in range(6):
            nc.tensor.matmul(
                ps_warm[:], zdum_sb[:, 0:128], zdum_sb[:],
                start=(k == 0), stop=(k == 5),
            )
        for mc in range(2):
            msl = slice(512 * mc, 512 * mc + 512)
            for ct in range(NT):
                ps = psa.tile([P, 512], f32, tag="psa", name="ps_a")
                for k in range(NT):
                    nc.tensor.matmul(
                        ps[:],
                        x_sb[:, k, 128 * ct:128 * ct + 128],
                        wq0t_sb[:, k, msl],
                        start=(k == 0), stop=(k == NT - 1),
                    )
                nc.vector.tensor_tensor(
                    x1t_sb[mc][:, ct, :], ps[:], bq0_sb[:, msl], Add)
        psa_cm.__exit__(None, None, None)
        wa_cm.__exit__(None, None, None)

        # E buffers reuse the A-phase input space (opened after wa closes).
        # 6 rotating buffers: exp head h writes e_tiles[h % 6]. The depth
        # matters: U of head h is emitted one score-block before exp h+6, so
        # the buffer's reads are fully retired before its next writer —
        # same-block reuse was observed to corrupt results on device.
        ep_cm = tc.tile_pool(name="ep", bufs=1)
        ep = ep_cm.__enter__()
        EB = 6
        e_tiles = [ep.tile([P, NT, N], bf16, name=f"e{i}_sb") for i in range(EB)]

        # ---------------- phase B chains (emitted interleaved with C) -------
        # PSUM layout (8 banks, no mid-stream handoffs — a score pool that
        # swaps banks mid-kernel inherits write-after-read waits against the
        # lagging exp stream): psw (2, bottom: U halves + proj1 chains, same
        # [128,512] shape, one rotating tag) | pst (4: scores, alive until
        # the last exp) | psb (2: B chains, also hosts the split p1(3)
        # accumulators at the tail).
        psw_cm = tc.tile_pool(name="psw", bufs=2, space="PSUM")
        pst_cm = tc.tile_pool(name="pst", bufs=2, space="PSUM")
        psb_cm = tc.tile_pool(name="psb", bufs=2, space="PSUM")
        psw = psw_cm.__enter__()
        pst = [pst_cm.__enter__()]
        psb = psb_cm.__enter__()

        # dt 0-3: q head pair dt; dt 4-7: k head pair dt-4
        def qk_chain(dt, nch):
            nsl = slice(512 * nch, 512 * nch + 512)
            ps = psb.tile([P, 512], f32, tag="psb", name="ps_b")
            for k in range(NT):
                nc.tensor.matmul(
                    ps[:],
                    wqk1t_sb[:, k, 128 * dt:128 * dt + 128],
                    x1t_sb[nch][:, k, :],
                    start=(k == 0), stop=(k == NT - 1),
                )
            dst = qi_sb[dt] if dt < 4 else ki_sb[dt - 4]
            nc.vector.tensor_scalar_add(
                dst[:, 0, nsl], ps[:], bqk1_sb[:, dt:dt + 1])

        def v_chain(mt):
            ps = psb.tile([P, 512], f32, tag="psb", name="ps_v")
            for k in range(NT):
                nc.tensor.matmul(
                    ps[:],
                    x1t_sb[mt // 4][:, k, 128 * (mt % 4):128 * (mt % 4) + 128],
                    wv1t_sb[:, k, :],
                    start=(k == 0), stop=(k == NT - 1),
                )
            vdst = vpad_sb[:, mt, :].rearrange(
                "p (h j) -> p h j", h=HPC)[:, :, 0:HD]
            vsrc = ps[:].rearrange("p (h j) -> p h j", h=HPC)
            bsrc = bv1_sb[:].rearrange("p (h j) -> p h j", h=HPC)
            nc.vector.tensor_tensor(vdst, vsrc, bsrc, Add)

        # ---------------- phase C helpers -----------------------------------
        def sc_half(h, mt, nch):
            # single [128,512] score chunk with its own exp, staged in the
            # (still idle) psw pool — costs an extra ACT init but lets head
            # 0's first exp fire one qk chain earlier, opening the ACT
            # stream sooner
            t, pr = h // 2, slice(64 * (h % 2), 64 * (h % 2) + 64)
            nsl = slice(512 * nch, 512 * nch + 512)
            st = psw.tile([P, 512], f32, tag="w", name="ps_st0")
            nc.tensor.matmul(
                st[:],
                ki_sb[t][pr, :, 128 * mt:128 * mt + 128],
                qi_sb[t][pr, :, nsl],
                start=True, stop=True,
                perf_mode=DoubleRow,
            )
            nc.scalar.activation(
                e_tiles[h % EB][:, mt, nsl], st[:], Exp, scale=SCALE)

        def sc_chunk(h, mt):
            t, pr = h // 2, slice(64 * (h % 2), 64 * (h % 2) + 64)
            st = pst[0].tile([P, N], f32, tag="st", name="ps_st")
            for nch in range(2):
                nsl = slice(512 * nch, 512 * nch + 512)
                nc.tensor.matmul(
                    st[:, nsl],
                    ki_sb[t][pr, :, 128 * mt:128 * mt + 128],
                    qi_sb[t][pr, :, nsl],
                    start=True, stop=True,
                    perf_mode=DoubleRow,
                )
            nc.scalar.activation(
                e_tiles[h % EB][:, mt, :], st[:], Exp, scale=SCALE)

        def u_half(h, half):
            u = psw.tile([P, 512], f32, tag="w", name="ps_u")
            et = e_tiles[h % EB]
            for jj in range(4):
                nt = 4 * half + jj
                usl = slice(65 * jj, 65 * jj + 65)
                for mt in range(NT):
                    nc.tensor.matmul(
                        u[:, usl],
                        et[:, mt, 128 * nt:128 * nt + 128],
                        vpad_sb[:, mt, VW * h:VW * h + VW],
                        start=(mt == 0), stop=(mt == NT - 1),
                    )
            for jj in range(4):
                nt = 4 * half + jj
                r = rp.tile([P, 1], f32, tag="r", name="r_sb")
                nc.vector.reciprocal(r[:], u[:, 65 * jj + 64:65 * jj + 65])
                nc.vector.tensor_scalar_mul(
                    o_sb[:, nt, h, :], u[:, 65 * jj:65 * jj + 64], r[:])

        def p1_chain(pair, mh):
            ps = psb.tile([P, 512], f32, tag="psb", name="ps_y")
            for k in range(NT):
                nc.tensor.matmul(
                    ps[:],
                    o_sb[:, k, 2 * pair:2 * pair + 2, :],
                    wp0t_sb[:, k, 512 * mh:512 * mh + 512],
                    start=(k == 0), stop=(k == NT - 1),
                )
            nc.vector.tensor_copy(
                y1t_sb[:, pair, 512 * mh:512 * mh + 512], ps[:])

        # ---------------- interleaved B + C schedule ------------------------
        # head h's scores need q pair chain dt=h//2 and k pair chain 4+h//2
        # (k nch=0 covers score mt 0-3, nch=1 mt 4-7). U of head h is emitted
        # in score block h+4 so exp h+5 (EB=5 rotation, same E buffer) never
        # waits on pending reads. proj1 of pair p follows U of head 2p+1.
        # keep ACT fed from the first chunk on: head 0's mt 0-3 go as
        # [128,512] half-chunks so the first exp only waits on two qk
        # chains; head 1's mt 0-3 (needing just ki[0] nch=0) fill the
        qk_chain(0, 0)
        qk_chain(4, 0)
        sc_half(0, 0, 0)
        sc_half(0, 1, 0)
        qk_chain(0, 1)
        sc_half(0, 2, 0)
        sc_half(0, 3, 0)
        sc_half(0, 0, 1)
        sc_half(0, 1, 1)
        qk_chain(4, 1)
        sc_half(0, 2, 1)
        sc_half(0, 3, 1)
        for mt in range(4):
            sc_chunk(1, mt)
        for mt in range(4, NT):
            sc_chunk(0, mt)

        # qk chains go as early as possible — from block 2 on, exp is near
        # the critical path and sc h2..h7 are gated only by their qk chains.
        # v chains only gate U h0 (block 5), so most fill block 4.
        fillers = {
            1: [lambda: qk_chain(1, 0), lambda: qk_chain(1, 1),
                lambda: qk_chain(5, 0), lambda: qk_chain(5, 1)],
            2: [lambda: qk_chain(2, 0), lambda: qk_chain(2, 1),
                lambda: qk_chain(6, 0), lambda: qk_chain(6, 1)],
            3: [lambda: qk_chain(3, 0), lambda: qk_chain(3, 1),
                lambda: qk_chain(7, 0), lambda: qk_chain(7, 1)],
            4: [lambda: v_chain(2), lambda: v_chain(3),
                lambda: v_chain(4), lambda: v_chain(5)],
            5: [lambda: u_half(0, 0), lambda: u_half(0, 1),
                lambda: u_half(1, 0), lambda: u_half(1, 1)],
            6: [lambda: u_half(2, 0), lambda: u_half(2, 1),
                lambda: p1_chain(0, 0), lambda: p1_chain(0, 1)],
            7: [lambda: u_half(3, 0), lambda: u_half(3, 1),
                lambda: u_half(4, 0), lambda: u_half(4, 1)],
        }

        for h in range(1, HPC):
            fl = fillers[h]
            if h == 1:
                # mt 0-3 already emitted upfront; one filler per remaining mt
                for mt in range(4, NT):
                    sc_chunk(h, mt)
                    fl[mt - 4]()
            else:
                for mt in range(NT):
                    sc_chunk(h, mt)
                    if mt % 2 == 1:
                        fl[mt // 2]()
            if h == 1:
                v_chain(0)
            elif h == 2:
                v_chain(1)
            elif h == 3:
                v_chain(6)
                v_chain(7)

        # C tail: remaining U and proj1. The critical chain after the last
        # exp is only: u7's mt7 steps -> norms -> p1(3) k4-7 -> proj2, so
        # p1(3)'s k0-3 steps (needing only u(7,0)'s n-tiles) are emitted
        # before u(7,1). The p1(3) accumulators live in psb (idle after the
        # last v chain) so they don't collide with the psw rotation.
        p1_chain(1, 0); p1_chain(1, 1)
        u_half(5, 0); u_half(5, 1)
        u_half(6, 0); u_half(6, 1)
        p1_chain(2, 0); p1_chain(2, 1)

        u_half(7, 0)
        ps3 = []
        for mh in range(2):
            ps = psb.tile([P, 512], f32, tag="psb", name="ps_y3")
            ps3.append(ps)
            for k in range(4):
                nc.tensor.matmul(
                    ps[:],
                    o_sb[:, k, 6:8, :],
                    wp0t_sb[:, k, 512 * mh:512 * mh + 512],
                    start=(k == 0), stop=False,
                )
        u_half(7, 1)

        def p1_finish(mh):
            for k in range(4, NT):
                nc.tensor.matmul(
                    ps3[mh][:],
                    o_sb[:, k, 6:8, :],
                    wp0t_sb[:, k, 512 * mh:512 * mh + 512],
                    start=False, stop=(k == NT - 1),
                )
            nc.scalar.copy(y1t_sb[:, 3, 512 * mh:512 * mh + 512], ps3[mh][:])

        p1_finish(0)
        p1_finish(1)
        psb_cm.__exit__(None, None, None)
        pst_cm.__exit__(None, None, None)

        # ---------------- proj2: out = Y1 @ Wp1_g^T -------------------------
        psp2_cm = tc.tile_pool(name="psp2", bufs=6, space="PSUM")
        psp2 = psp2_cm.__enter__()

        def p2_chain(mt, dch):
            dsl = slice(512 * dch, 512 * dch + 512)
            ps = psp2.tile([P, 512], f32, tag="p2", name="ps_o")
            for k in range(4):
                nc.tensor.matmul(
                    ps[:],
                    y1t_sb[:, k, 128 * mt:128 * mt + 128],
                    wp1t_sb[:, k, dsl],
                    start=(k == 0), stop=(k == 3),
                )
            ostage = rp.tile([P, 512], bf16, tag="ost", name="out_stage",
                             bufs=5)
            if dch == 0:
                nc.vector.tensor_copy(ostage[:], ps[:])
            else:
                nc.scalar.copy(ostage[:], ps[:])
            nc.sync.dma_start(out=out_d[mt, :, dsl], in_=ostage[:])

        for mt in range(NT):
            p2_chain(mt, 0)
            p2_chain(mt, 1)
        psp2_cm.__exit__(None, None, None)
        psw_cm.__exit__(None, None, None)

        ep_cm.__exit__(None, None, None)
        ab_cm.__exit__(None, None, None)
        wb_cm.__exit__(None, None, None)
        qkv_cm.__exit__(None, None, None)
        otp_cm.__exit__(None, None, None)
        wd_cm.__exit__(None, None, None)
        yp_cm.__exit__(None, None, None)
        rp_cm.__exit__(None, None, None)

    nc.compile()
    return nc


def _get_nc(reps=1):
    key = ("nc", reps)
    if key not in _CACHE:
        _CACHE[key] = _build(reps)
    return _CACHE[key]


def _in_maps(x, Wq0, bq0, Wq1, bq1, Wp0, bp0, Wp1, bp1):
    import ml_dtypes
    f = np.float32
    bf = ml_dtypes.bfloat16
    x = np.asarray(x, f)
    Wq0 = np.asarray(Wq0, f); bq0 = np.asarray(bq0, f)
    Wq1 = np.asarray(Wq1, f); bq1 = np.asarray(bq1, f)
    Wp0 = np.asarray(Wp0, f); Wp1 = np.asarray(Wp1, f)
    wq0t = np.ascontiguousarray(Wq0.T.reshape(NT, P, N)).astype(bf)
    wp0t = np.ascontiguousarray(Wp0.T.reshape(NT, P, N)).astype(bf)
    bq0r = np.ascontiguousarray(np.broadcast_to(bq0, (P, N)))
    maps = []
    for core in range(8):
        b, g = core // 2, core % 2
        qs = slice(512 * g, 512 * g + 512)
        ks = slice(1024 + 512 * g, 1024 + 512 * g + 512)
        vs = slice(2048 + 512 * g, 2048 + 512 * g + 512)
        wqk1 = np.concatenate([Wq1[qs], Wq1[ks]], 0)          # (1024 d', 1024 c)
        m = {
            "x_r": np.ascontiguousarray(x[b].reshape(NT, P, C)).astype(bf),
            "wq0t_r": wq0t,
            "wqk1t_r": np.ascontiguousarray(
                wqk1.T.reshape(NT, P, 1024)).astype(bf),
            "wv1t_r": np.ascontiguousarray(
                Wq1[vs].T.reshape(NT, P, 512)).astype(bf),
            "wp0t_r": wp0t,
            "wp1t_r": np.ascontiguousarray(
                Wp1[:, qs].T.reshape(4, P, 1024)).astype(bf),
            "bq0_rep": bq0r,
            "bqk1_t": np.ascontiguousarray(
                np.concatenate([bq1[qs], bq1[ks]]).reshape(8, P).T),
            "bv1_rep": np.ascontiguousarray(np.broadcast_to(bq1[vs], (P, 512))),
        }
        maps.append(m)
    return maps


def kernel(x, Wq0, bq0, Wq1, bq1, Wp0, bp0, Wp1, bp1):
    global LAST_RESULT
    import os

    # The SPMD execute path needs jax's axon PJRT backend; a harness that
    # pinned JAX_PLATFORMS=cpu (common for running the jax reference) would
    # otherwise hide the NeuronCores from this process.
    if "axon" not in os.environ.get("JAX_PLATFORMS", "axon"):
        os.environ.pop("JAX_PLATFORMS", None)
    # This container lacks antenv.axon_hooks, so the BASS_TRACE=1 NTFF path
    # in run_bass_kernel_spmd raises ModuleNotFoundError. Force tracing off
    # (a crash would otherwise replace a working run).
    os.environ["BASS_NEVER_TRACE"] = "1"
    from concourse.bass_utils import run_bass_kernel_spmd

    nc = _get_nc()
    maps = _in_maps(x, Wq0, bq0, Wq1, bq1, Wp0, bp0, Wp1, bp1)
    res = run_bass_kernel_spmd(nc, maps, list(range(8)))
    LAST_RESULT = res
    parts = [np.asarray(r["out_r"]).astype(np.float32).reshape(N, C)
             for r in res.results]
    f = np.float32
    bp0 = np.asarray(bp0, f); bp1 = np.asarray(bp1, f)
    Wp1 = np.asarray(Wp1, f)
    bias = np.outer(bp0, Wp1.sum(axis=1)) + bp1[None, :]
    out = np.stack(
        [parts[2 * b] + parts[2 * b + 1] + bias for b in range(B)], 0)
    return out.astype(f)
